# revision 29
# baseline (speedup 1.0000x reference)
"""Multi-head attention (B=4,S=2048,D=1024,H=16,dh=65) on 8 TRN2 NeuronCores.

Sharding: batch x head-half. Core c handles batch c//2 and heads
(c%2)*8..(c%2)*8+8 (P-slice of 520). Each core computes its QKV projections,
attention, and a partial out-projection; the host sums the two partials per
batch and adds bo.

Layout: Q/K projections are computed directly transposed (W stationary,
per-head M=65, N=512) so no PE transposes or PSUM round-trips are needed;
biases fold into the PSUM->SBUF copy as per-partition tensor_scalar adds.
Per head, A@V matmuls are emitted two rounds behind the score/exp/mask
chain and the interleaved work (Q-chunk projection, normalization,
out-projection, reciprocal pieces) is woven between score rounds as
"fillers" so the strict per-engine FIFOs never stall ACT or the PE at
head/qb boundaries. The sel8 normalization broadcast runs as a 1-pass
float32r matmul; output is written bf16 and upcast on the host. Softmax is
unnormalized (no max subtraction; score magnitudes are bounded) with the
row-sum harvested from a trailing ones-column in V.
"""

import math
import sys
from contextlib import ExitStack

import numpy as np
import ml_dtypes

sys.path.insert(0, "/opt/trn_rl_repo")

import concourse.bass as bass
import concourse.mybir as mybir
import concourse.tile as tile_mod
from concourse.bass_utils import run_bass_kernel_spmd
from concourse.vector_clock import ScopedClock

# ---------------------------------------------------------------------------
# Patch for this container's walrus build: it rejects instructions carrying
# more than one semaphore wait ("Too many sync wait commands"), but Tile's
# wait assigner freely attaches several. Split excess waits onto bass_nofuse
# InstNoOp carriers on the same engine, committed immediately before the
# instruction (same-engine program order => over-synchronization only).
# ---------------------------------------------------------------------------
_MAX_WAITS = 1

_orig_commit = tile_mod.TileContext._commit_instruction


def _split_waits(self, inst, commit):
    si = inst.sync_info
    if si is None or len(si.on_wait) <= _MAX_WAITS:
        return
    waits = list(si.on_wait)
    sem_w = [w for w in waits if getattr(w, "sync_type", "semaphore") == "semaphore"]
    other_w = [w for w in waits if getattr(w, "sync_type", "semaphore") != "semaphore"]
    keep_budget = _MAX_WAITS - len(other_w)
    if keep_budget < 0:
        return
    keep = other_w + (sem_w[-keep_budget:] if keep_budget > 0 else [])
    excess = sem_w[: len(sem_w) - max(keep_budget, 0)]
    if not excess:
        return
    for i, w in enumerate(excess):
        nop = mybir.InstNoOp(
            name=f"{inst.name}-sw{i}",
            sync_info=mybir.SyncInfo(on_wait=[w], on_update=[]),
            bass_nofuse=True,
            engine=inst.engine,
        )
        commit(nop)
    inst.sync_info = mybir.SyncInfo(on_wait=keep, on_update=list(si.on_update))


def _patched_commit(self, inst, lazy_reg_writes: bool = True):
    if inst.engine != mybir.EngineType.Unassigned:
        _split_waits(self, inst, lambda n: _orig_commit(self, n, False))
    return _orig_commit(self, inst, lazy_reg_writes)


def _patched_drain_and_barrier(self, tick_clock, wait_clock):
    drain_inst = self.nc.sync.drain()
    wait_clock.add_sem_waits(
        drain_inst.ins, ScopedClock({None: tick_clock.global_clock})
    )
    si = drain_inst.ins.sync_info
    if si is not None and len(si.on_wait) > _MAX_WAITS:
        waits = list(si.on_wait)
        drain_inst.ins.sync_info = mybir.SyncInfo(
            on_wait=waits[:_MAX_WAITS], on_update=list(si.on_update)
        )
        for w in waits[_MAX_WAITS:]:
            n = self.nc.sync.nop(nofuse=True)
            n.ins.sync_info = mybir.SyncInfo(on_wait=[w], on_update=[])
    self.nc.all_engine_barrier()
    popped = self.nc._tile_sem_poison_stack.pop()
    assert popped is self._sem_poison
    self.nc.clear_and_free_semaphores(list(self.sems.allocated().values()))
    self.nc.all_engine_barrier()


tile_mod.TileContext._commit_instruction = _patched_commit
tile_mod.TileContext._drain_and_barrier = _patched_drain_and_barrier

# ---------------------------------------------------------------------------

B, S, D, H = 4, 2048, 1024, 16
DH = D // H + 1          # 65
P = H * DH               # 1040
HPC = H // 2             # heads per core
PC = HPC * DH            # 520, per-core P slice
N_CORES = 8

MT = S // 128            # 16 row blocks / k tiles
KT = 16                  # k tiles per attention
QB = 4                   # q blocks of 512
QW = 512
RKT = 2                  # k-tiles per score round (2 banks, double-buffered)
NR = KT // RKT           # 8 rounds

F32 = mybir.dt.float32
F32R = mybir.dt.float32r
BF16 = mybir.dt.bfloat16
BF = ml_dtypes.bfloat16

_BUILT = {}


def _build_nc():
    nc = bass.Bass("TRN2", target_bir_lowering=False, debug=False,
                   num_devices=N_CORES)

    xq_d = nc.dram_tensor("xq", [D, S], BF16, kind="ExternalInput").ap()
    xk_d = nc.dram_tensor("xk", [D, S], BF16, kind="ExternalInput").ap()
    xv_d = nc.dram_tensor("xv", [D, S], BF16, kind="ExternalInput").ap()
    # maskH[qb, p, j*QW+q] = maskT[j*128+p, qb*512+q] (multiplicative 0/1)
    mh = nc.dram_tensor("maskH", [QB, 128, KT * QW], BF16,
                        kind="ExternalInput").ap()
    wq_d = nc.dram_tensor("wqT", [D, PC], BF16, kind="ExternalInput").ap()
    wk_d = nc.dram_tensor("wkT", [D, PC], BF16, kind="ExternalInput").ap()
    wv_d = nc.dram_tensor("wvT", [D, PC], BF16, kind="ExternalInput").ap()
    bqT_d = nc.dram_tensor("bqT", [DH, HPC], F32, kind="ExternalInput").ap()
    bkT_d = nc.dram_tensor("bkT", [DH, HPC], F32, kind="ExternalInput").ap()
    bv_d = nc.dram_tensor("bv", [1, PC], BF16, kind="ExternalInput").ap()
    wo_d = nc.dram_tensor("woT", [PC, D], BF16, kind="ExternalInput").ap()
    sel8_d = nc.dram_tensor("sel8", [HPC, HPC * DH], F32R,
                            kind="ExternalInput").ap()
    out = nc.dram_tensor("out", [S, D], BF16, kind="ExternalOutput").ap()

    # packed 128-row k-tile ranges of the 520-row concatT / WoT
    PKT = [(0, 128), (128, 256), (256, 384), (384, 512), (512, 520)]

    with tile_mod.TileContext(nc) as tc:
        with tc.tile_pool(name="const", bufs=1) as pconst, \
             tc.tile_pool(name="qkT", bufs=1) as pqkT, \
             tc.tile_pool(name="vh", bufs=MT) as pvh, \
             tc.tile_pool(name="pwq", bufs=8) as pwq, \
             tc.tile_pool(name="pbq", bufs=1) as pbq, \
             tc.tile_pool(name="pxc", bufs=10) as pxc:

            ones_col = pconst.tile([1, 128], BF16, tag="ones")
            nc.gpsimd.memset(ones_col[:], 1.0)
            sel8 = pconst.tile([HPC, HPC * DH], F32R, tag="sel8")

            # [65, proj(q=0,k=1), head, S] -- dh on partitions
            qkT = pqkT.tile([DH, 2, HPC, S], BF16, tag="qkT")
            # v k-tiles with trailing ones column: [128, head, 65+1]
            vh = [pvh.tile([128, HPC, DH + 1], BF16, tag="vh", name=f"vh{j}")
                  for j in range(MT)]
            for j in range(MT):
                nc.gpsimd.memset(vh[j][:, :, DH:DH + 1], 1.0)

            def load_xq_chunk(qc):
                xts = []
                for d in range(8):
                    xt = pxc.tile([128, QW], BF16, tag="xqc",
                                  name=f"xqc{qc}_{d}")
                    nc.sync.dma_start(
                        xt[:], xq_d[d * 128:(d + 1) * 128,
                                    qc * QW:(qc + 1) * QW])
                    xts.append(xt)
                return xts

            def tproj_head(pj, pi, h, sblk, xts, wts, bT, xoff=0):
                """Transposed projection: qkT[0:65, pi, h, sblk] block."""
                ps = pj.tile([128, QW], F32, tag="psp",
                             name=f"tp{pi}_{h}_{sblk}")
                hs = slice(h * DH, (h + 1) * DH)
                xlo = sblk * QW - xoff
                for d in range(8):
                    nc.tensor.matmul(ps[0:DH, :], wts[d][:, hs],
                                     xts[d][:, xlo:xlo + QW],
                                     start=(d == 0), stop=(d == 7))
                nc.vector.tensor_scalar_add(
                    qkT[:, pi, h, sblk * QW:(sblk + 1) * QW], ps[0:DH, :],
                    bT[:, h:h + 1])

            # ---------------- phase A: projections --------------------------
            with tc.tile_pool(name="pxk", bufs=8) as pxk, \
                 tc.tile_pool(name="pxv", bufs=8) as pxv, \
                 tc.tile_pool(name="pwkv", bufs=9) as pwkv, \
                 tc.tile_pool(name="pbkv", bufs=2) as pbkv, \
                 tc.tile_pool(name="pj", bufs=5, space="PSUM") as pj, \
                 tc.tile_pool(name="psV", bufs=3, space="PSUM") as psV:

                xk_t, xv_t, wk_t, wv_t = [], [], [], []
                for d in range(8):
                    xt = pxk.tile([128, S], BF16, tag="xk", name=f"xk{d}")
                    eng = nc.sync if d % 2 == 0 else nc.scalar
                    eng.dma_start(xt[:], xk_d[d * 128:(d + 1) * 128, :])
                    xk_t.append(xt)
                for d in range(8):
                    wt = pwkv.tile([128, PC], BF16, tag="wk", name=f"wk{d}")
                    nc.gpsimd.dma_start(wt[:], wk_d[d * 128:(d + 1) * 128, :])
                    wk_t.append(wt)
                bkT = pbkv.tile([DH, HPC], F32, tag="bkT")
                nc.gpsimd.dma_start(bkT[:], bkT_d[:])
                for d in range(8):
                    xt = pxv.tile([128, S], BF16, tag="xv", name=f"xv{d}")
                    eng = nc.sync if d % 2 == 0 else nc.scalar
                    eng.dma_start(xt[:], xv_d[d * 128:(d + 1) * 128, :])
                    xv_t.append(xt)
                for d in range(8):
                    wt = pwkv.tile([128, HPC, DH], BF16, tag="wv",
                                   name=f"wv{d}")
                    nc.gpsimd.dma_start(
                        wt[:], wv_d[d * 128:(d + 1) * 128, :])
                    wv_t.append(wt)
                bv_t = pbkv.tile([1, HPC, DH], BF16, tag="bv")
                nc.gpsimd.dma_start(bv_t[:], bv_d[:])
                nc.gpsimd.dma_start(sel8[:], sel8_d[:])

                wq_t = []
                for d in range(8):
                    wt = pwq.tile([128, PC], BF16, tag="wq", name=f"wq{d}")
                    nc.scalar.dma_start(wt[:],
                                        wq_d[d * 128:(d + 1) * 128, :])
                    wq_t.append(wt)
                bqT = pbq.tile([DH, HPC], F32, tag="bqT")
                nc.gpsimd.dma_start(bqT[:], bqT_d[:])
                xq_c0 = load_xq_chunk(0)

                # K projection (transposed), then Q chunk 0
                for h in range(HPC):
                    for sblk in range(4):
                        tproj_head(pj, 1, h, sblk, xk_t, wk_t, bkT)
                for h in range(HPC):
                    tproj_head(pj, 0, h, 0, xq_c0, wq_t, bqT, xoff=0)

                # V projection (row-major into vh, bias via ones-row matmul)
                for m in range(MT):
                    for half in range(2):
                        hs = half * 4
                        ps = psV.tile([128, 4, DH], F32, tag="psV",
                                      name=f"pv{m}_{half}")
                        nc.tensor.matmul(ps[:], ones_col[0:1, :],
                                         bv_t[0:1, hs:hs + 4, :],
                                         start=True, stop=False)
                        for d in range(8):
                            nc.tensor.matmul(
                                ps[:], xv_t[d][:, m * 128:(m + 1) * 128],
                                wv_t[d][:, hs:hs + 4, :],
                                start=False, stop=(d == 7))
                        nc.vector.tensor_copy(vh[m][:, hs:hs + 4, 0:DH],
                                              ps[:])

            # ---------------- phase B: attention + out-projection -----------
            with tc.tile_pool(name="pm", bufs=4) as pm, \
                 tc.tile_pool(name="pp", bufs=6) as pp, \
                 tc.tile_pool(name="pc", bufs=1) as pc, \
                 tc.tile_pool(name="pwo", bufs=1) as pwo, \
                 tc.tile_pool(name="po", bufs=2) as po, \
                 tc.tile_pool(name="pt2", bufs=4) as pt2, \
                 tc.tile_pool(name="psS", bufs=2, space="PSUM") as psS, \
                 tc.tile_pool(name="psA", bufs=2, space="PSUM") as psA, \
                 tc.tile_pool(name="psp", bufs=2, space="PSUM") as psp:

                # packed concatT: 128-row tiles covering rows 0..520
                ccp = [pc.tile([b - a, S], BF16, tag=f"ccp{i}",
                               name=f"ccp{i}")
                       for i, (a, b) in enumerate(PKT)]
                wop = []
                for i, (a, b) in enumerate(PKT):
                    w = pwo.tile([b - a, D], BF16, tag=f"wop{i}",
                                 name=f"wop{i}")
                    nc.scalar.dma_start(w[:], wo_d[a:b, :])
                    wop.append(w)

                inv_sqrt = 1.0 / math.sqrt(float(DH))
                state = {}
                rnd = [0]  # global round counter for DVE/Pool mask split

                def load_masks(qb):
                    mts = []
                    for hf in range(2):
                        mt = pm.tile([128, KT // 2, QW], BF16, tag="mask",
                                     name=f"mask{qb}_{hf}")
                        nc.scalar.dma_start(
                            mt[:], mh[qb, :, hf * (KT // 2) * QW:
                                      (hf + 1) * (KT // 2) * QW])
                        mts.append(mt)
                    return mts

                def attn_head(qb, h, mts, rsall, uovs, fillers=()):
                    """One head's rounds; fillers are emitted between rounds
                    so interleaved PE work lands in exp/mask wait gaps
                    instead of starving ACT at head boundaries."""
                    fillers = list(fillers)
                    ov = psA.tile([128, QW], F32, tag="psA",
                                  name=f"ov{qb}_{h}")
                    pts = []

                    def av_round(r):
                        pt = pts[r]
                        for jj in range(RKT):
                            j = r * RKT + jj
                            nc.tensor.matmul(
                                ov[0:DH + 1, :], vh[j][:, h, :],
                                pt[:, jj, :],
                                start=(j == 0), stop=(j == KT - 1))

                    for r in range(NR):
                        ss = psS.tile([128, RKT, QW], F32, tag="psS",
                                      name=f"ss{qb}_{h}_{r}")
                        for jj in range(RKT):
                            j = r * RKT + jj
                            nc.tensor.matmul(
                                ss[:, jj, :],
                                qkT[:, 1, h, j * 128:(j + 1) * 128],
                                qkT[:, 0, h, qb * QW:(qb + 1) * QW],
                                start=True, stop=True)
                        pt = pp.tile([128, RKT, QW], BF16, tag="pT",
                                     name=f"pt{qb}_{h}_{r}")
                        nc.scalar.activation(
                            pt[:], ss[:],
                            mybir.ActivationFunctionType.Exp,
                            scale=inv_sqrt)
                        mt = mts[r // (NR // 2)]
                        rr = r % (NR // 2)
                        nc.vector.tensor_mul(
                            pt[:], pt[:], mt[:, rr * RKT:(rr + 1) * RKT, :])
                        pts.append(pt)
                        if r >= 2:
                            av_round(r - 2)
                        if r >= 2 and fillers:
                            fillers.pop(0)()
                    while fillers:
                        fillers.pop(0)()
                    av_round(NR - 2)
                    av_round(NR - 1)
                    uov = pt2.tile([DH + 1, QW], BF16, tag="uov",
                                   name=f"uov{qb}_{h}", bufs=12)
                    nc.vector.tensor_copy(uov[:], ov[0:DH + 1, :])
                    nc.sync.dma_start(rsall[h:h + 1, :],
                                      uov[DH:DH + 1, :])
                    uovs.append(uov)

                def tproj_pieces(pi, hh, sblk, xts, bT, xoff):
                    box = {}

                    def p1():
                        ps = psp.tile([128, QW], F32, tag="psp",
                                      name=f"tpp{pi}_{hh}_{sblk}")
                        box["ps"] = ps
                        hsl = slice(hh * DH, (hh + 1) * DH)
                        xlo = sblk * QW - xoff
                        for d in range(4):
                            nc.tensor.matmul(ps[0:DH, :], wq_t[d][:, hsl],
                                             xts[d][:, xlo:xlo + QW],
                                             start=(d == 0), stop=False)

                    def p2():
                        ps = box["ps"]
                        hsl = slice(hh * DH, (hh + 1) * DH)
                        xlo = sblk * QW - xoff
                        for d in range(4, 8):
                            nc.tensor.matmul(ps[0:DH, :], wq_t[d][:, hsl],
                                             xts[d][:, xlo:xlo + QW],
                                             start=False, stop=(d == 7))
                        nc.vector.tensor_scalar_add(
                            qkT[:, pi, hh, sblk * QW:(sblk + 1) * QW],
                            ps[0:DH, :], bT[:, hh:hh + 1])

                    return [p1, p2]

                def outproj_pieces(m):
                    box = {}

                    def pn(n):
                        def run():
                            if n == 0:
                                box["osb"] = po.tile([128, D], BF16,
                                                     tag="osb",
                                                     name=f"osb{m}")
                            osb = box["osb"]
                            ps = psp.tile([128, QW], F32, tag="psp",
                                          name=f"psop{m}_{n}")
                            for i, (a, b) in enumerate(PKT):
                                nc.tensor.matmul(
                                    ps[:], ccp[i][:, m * 128:(m + 1) * 128],
                                    wop[i][:, n * QW:(n + 1) * QW],
                                    start=(i == 0),
                                    stop=(i == len(PKT) - 1))
                            nc.vector.tensor_copy(
                                osb[:, n * QW:(n + 1) * QW], ps[:])
                            if n == 1:
                                nc.scalar.dma_start(
                                    out[m * 128:(m + 1) * 128, :], osb[:])
                        return run

                    return [pn(0), pn(1)]

                def compute_recip(qb, lo, hi, c0=0, c1=QW):
                    st = state[qb]
                    if "rcall" not in st:
                        st["rcall"] = pt2.tile([HPC, QW], F32R, tag="rcall",
                                               name=f"rcall{qb}", bufs=2)
                    with nc.allow_low_precision(
                            reason="f32r reciprocal feeds 1-pass f32r "
                                   "broadcast matmul; 2^-11 rel is ample"):
                        nc.vector.reciprocal(st["rcall"][lo:hi, c0:c1],
                                             st["rsall"][lo:hi, c0:c1])

                def norm_heads(qb, hs):
                    st = state[qb]
                    if "rcall" not in st:
                        compute_recip(qb, 0, HPC)
                    rcall = st["rcall"]
                    for h in hs:
                        rbp = psp.tile([128, QW], F32, tag="psp",
                                       name=f"rbp{qb}_{h}")
                        nc.tensor.matmul(rbp[0:DH, :],
                                         sel8[:, h * DH:(h + 1) * DH],
                                         rcall[:],
                                         start=True, stop=True)
                        cch = pt2.tile([DH, QW], BF16, tag="cch",
                                       name=f"cch{qb}_{h}", bufs=2)
                        nc.vector.tensor_mul(cch[:], rbp[0:DH, :],
                                             st["uovs"][h][0:DH, :])
                        # pack into 128-row concatT tiles (DMA shifts rows)
                        r0 = h * DH
                        for i, (a, b) in enumerate(PKT):
                            lo, hi = max(r0, a), min(r0 + DH, b)
                            if lo < hi:
                                nc.gpsimd.dma_start(
                                    ccp[i][lo - a:hi - a,
                                           qb * QW:(qb + 1) * QW],
                                    cch[lo - r0:hi - r0, :])

                def outproj_m(m):
                    osb = po.tile([128, D], BF16, tag="osb")
                    for n in range(2):
                        ps = psp.tile([128, QW], F32, tag="psp",
                                      name=f"psop{m}_{n}")
                        for i, (a, b) in enumerate(PKT):
                            nc.tensor.matmul(
                                ps[:], ccp[i][:, m * 128:(m + 1) * 128],
                                wop[i][:, n * QW:(n + 1) * QW],
                                start=(i == 0), stop=(i == len(PKT) - 1))
                        nc.vector.tensor_copy(osb[:, n * QW:(n + 1) * QW],
                                              ps[:])
                    nc.sync.dma_start(out[m * 128:(m + 1) * 128, :],
                                      osb[:])

                mts = load_masks(0)
                for qb in range(QB):
                    nmts = load_masks(qb + 1) if qb < QB - 1 else None
                    rsall = pt2.tile([HPC, QW], BF16, tag="rsall",
                                     name=f"rsall{qb}", bufs=2)
                    uovs = []
                    state[qb] = {"rsall": rsall, "uovs": uovs}
                    last = qb == QB - 1
                    for h in range(HPC):
                        if last and h == 5:
                            compute_recip(qb, 0, 5)
                        fill = []
                        if qb > 0 and h == 0:
                            for ci in range(1, 4):
                                fill.append(
                                    lambda q_=qb - 1, c_=ci: compute_recip(
                                        q_, 0, HPC, c_ * QW // 4,
                                        (c_ + 1) * QW // 4))
                            if qb >= 2:
                                fill += outproj_pieces((qb - 2) * 4 + 3)
                        if qb < QB - 1:
                            if h == 0:
                                state["xq"] = load_xq_chunk(qb + 1)
                            fill += tproj_pieces(0, h, qb + 1, state["xq"],
                                                 bqT, (qb + 1) * QW)
                        if qb > 0:
                            if 1 <= h <= 4:
                                fill.append(
                                    lambda q_=qb - 1, a_=2 * (h - 1):
                                    norm_heads(q_, [a_]))
                                fill.append(
                                    lambda q_=qb - 1, a_=2 * (h - 1) + 1:
                                    norm_heads(q_, [a_]))
                            elif h >= 5:
                                fill += outproj_pieces(
                                    (qb - 1) * 4 + (h - 5))
                                if last and h == 5:
                                    fill += outproj_pieces((QB - 2) * 4 + 3)
                        if last and h == 6:
                            for hh in range(0, 3):
                                fill.append(
                                    lambda a_=hh: norm_heads(qb, [a_]))
                        elif last and h == 7:
                            fill.insert(0, lambda: compute_recip(
                                qb, 0, 7, QW // 2, QW))
                            fill.insert(0, lambda: compute_recip(
                                qb, 0, 7, 0, QW // 2))
                            for hh in range(3, 7):
                                fill.append(
                                    lambda a_=hh: norm_heads(qb, [a_]))
                        attn_head(qb, h, mts, rsall, uovs, fill)
                    if not last:
                        # first reciprocal piece at the boundary; the rest
                        # interleave as fillers so neither PE nor DVE stalls
                        compute_recip(qb, 0, HPC, 0, QW // 4)
                    mts = nmts
                compute_recip(QB - 1, 0, HPC)
                norm_heads(QB - 1, range(7, HPC))
                for m in range((QB - 1) * 4, QB * 4):
                    for p in outproj_pieces(m):
                        p()

    return nc


F8NP = ml_dtypes.float8_e4m3


def pack8(xT, scale):
    """[D, N] -> [4, 128, 2, N] fp8 with d-chunk pairs interleaved."""
    D_, N_ = xT.shape
    y = (xT * scale).reshape(4, 2, 128, N_).transpose(0, 2, 1, 3)
    assert np.abs(y).max() < 240.0
    return np.ascontiguousarray(y).astype(F8NP)


def pack8w(wT, scale):
    """[D, PC] -> [4, 128, 2, HPC, 80] fp8, heads padded 65 -> 80."""
    D_, _ = wT.shape
    y = np.zeros((4, 2, 128, HPC, 80), np.float32)
    w = (wT * scale).reshape(4, 2, 128, HPC, DH)
    assert np.abs(w).max() < 240.0
    y[..., 0:DH] = w
    return np.ascontiguousarray(y.transpose(0, 2, 1, 3, 4)).astype(F8NP)


def _prep_inputs(q, k, v, mask, Wq, bqv, Wk, bkv, Wv, bvv, Wo):
    """Per-core input maps (numpy, host-side shard + cast)."""
    in_maps = []
    sel8 = np.zeros((HPC, HPC * DH), np.float32)
    for h in range(HPC):
        sel8[h, h * DH:(h + 1) * DH] = 1.0
    mask_h = {}
    for b in range(B):
        mt = (mask[b, 0] != 0).astype(np.float32).T  # [k, q]
        m4 = mt.reshape(KT, 128, QB, QW).transpose(2, 1, 0, 3)
        mask_h[b] = np.ascontiguousarray(m4.reshape(QB, 128, KT * QW)).astype(BF)
    for c in range(N_CORES):
        b, hh = c // 2, c % 2
        sl = slice(hh * PC, (hh + 1) * PC)
        in_maps.append({
            "xq": np.ascontiguousarray(q[b].T).astype(BF),
            "xk": np.ascontiguousarray(k[b].T).astype(BF),
            "xv": np.ascontiguousarray(v[b].T).astype(BF),
            "maskH": mask_h[b],
            "wqT": np.ascontiguousarray(Wq[sl, :].T).astype(BF),
            "wkT": np.ascontiguousarray(Wk[sl, :].T).astype(BF),
            "wvT": np.ascontiguousarray(Wv[sl, :].T).astype(BF),
            "bqT": np.ascontiguousarray(
                bqv[sl].reshape(HPC, DH).T).astype(np.float32),
            "bkT": np.ascontiguousarray(
                bkv[sl].reshape(HPC, DH).T).astype(np.float32),
            "bv": bvv[sl].reshape(1, PC).astype(BF),
            "woT": np.ascontiguousarray(Wo[:, sl].T).astype(BF),
            "sel8": sel8,
        })
    return in_maps


def run_sharded(in_maps, **kwargs):
    if "nc" not in _BUILT:
        _BUILT["nc"] = _build_nc()
    return run_bass_kernel_spmd(_BUILT["nc"], in_maps,
                                core_ids=list(range(N_CORES)), **kwargs)


def kernel(q, k, v, mask, Wq, bq, Wk, bk, Wv, bv, Wo, bo):
    q = np.asarray(q, np.float32)
    k = np.asarray(k, np.float32)
    v = np.asarray(v, np.float32)
    mask = np.asarray(mask)
    in_maps = _prep_inputs(q, k, v, mask,
                           np.asarray(Wq, np.float32), np.asarray(bq, np.float32),
                           np.asarray(Wk, np.float32), np.asarray(bk, np.float32),
                           np.asarray(Wv, np.float32), np.asarray(bv, np.float32),
                           np.asarray(Wo, np.float32))
    res = run_sharded(in_maps)
    bo32 = np.asarray(bo, np.float32)
    out = np.empty((B, S, D), np.float32)
    for b in range(B):
        out[b] = (res.results[2 * b]["out"].astype(np.float32)
                  + res.results[2 * b + 1]["out"].astype(np.float32) + bo32)
    return out


# revision 30
# speedup vs baseline: 1.0305x; 1.0305x over previous
"""Multi-head attention (B=4,S=2048,D=1024,H=16,dh=65) on 8 TRN2 NeuronCores.

Sharding: batch x head-half. Core c handles batch c//2 and heads
(c%2)*8..(c%2)*8+8 (P-slice of 520). Each core computes its QKV projections,
attention, and a partial out-projection; the host sums the two partials per
batch and adds bo.

Layout: Q/K projections are computed directly transposed (W stationary,
per-head M=65, N=512) so no PE transposes or PSUM round-trips are needed;
biases fold into the PSUM->SBUF copy as per-partition tensor_scalar adds.
Per head, A@V matmuls are emitted two rounds behind the score/exp/mask
chain and the interleaved work (Q-chunk projection, normalization,
out-projection, reciprocal pieces) is woven between score rounds as
"fillers" so the strict per-engine FIFOs never stall ACT or the PE at
head/qb boundaries. The sel8 normalization broadcast runs as a 1-pass
float32r matmul; output is written bf16 and upcast on the host. Softmax is
unnormalized (no max subtraction; score magnitudes are bounded) with the
row-sum harvested from a trailing ones-column in V.
"""

import math
import sys
from contextlib import ExitStack

import numpy as np
import ml_dtypes

sys.path.insert(0, "/opt/trn_rl_repo")

import concourse.bass as bass
import concourse.mybir as mybir
import concourse.tile as tile_mod
from concourse.bass_utils import run_bass_kernel_spmd
from concourse.vector_clock import ScopedClock

# ---------------------------------------------------------------------------
# Patch for this container's walrus build: it rejects instructions carrying
# more than one semaphore wait ("Too many sync wait commands"), but Tile's
# wait assigner freely attaches several. Split excess waits onto bass_nofuse
# InstNoOp carriers on the same engine, committed immediately before the
# instruction (same-engine program order => over-synchronization only).
# ---------------------------------------------------------------------------
_MAX_WAITS = 1

_orig_commit = tile_mod.TileContext._commit_instruction


def _split_waits(self, inst, commit):
    si = inst.sync_info
    if si is None or len(si.on_wait) <= _MAX_WAITS:
        return
    waits = list(si.on_wait)
    sem_w = [w for w in waits if getattr(w, "sync_type", "semaphore") == "semaphore"]
    other_w = [w for w in waits if getattr(w, "sync_type", "semaphore") != "semaphore"]
    keep_budget = _MAX_WAITS - len(other_w)
    if keep_budget < 0:
        return
    keep = other_w + (sem_w[-keep_budget:] if keep_budget > 0 else [])
    excess = sem_w[: len(sem_w) - max(keep_budget, 0)]
    if not excess:
        return
    for i, w in enumerate(excess):
        nop = mybir.InstNoOp(
            name=f"{inst.name}-sw{i}",
            sync_info=mybir.SyncInfo(on_wait=[w], on_update=[]),
            bass_nofuse=True,
            engine=inst.engine,
        )
        commit(nop)
    inst.sync_info = mybir.SyncInfo(on_wait=keep, on_update=list(si.on_update))


def _patched_commit(self, inst, lazy_reg_writes: bool = True):
    if inst.engine != mybir.EngineType.Unassigned:
        _split_waits(self, inst, lambda n: _orig_commit(self, n, False))
    return _orig_commit(self, inst, lazy_reg_writes)


def _patched_drain_and_barrier(self, tick_clock, wait_clock):
    drain_inst = self.nc.sync.drain()
    wait_clock.add_sem_waits(
        drain_inst.ins, ScopedClock({None: tick_clock.global_clock})
    )
    si = drain_inst.ins.sync_info
    if si is not None and len(si.on_wait) > _MAX_WAITS:
        waits = list(si.on_wait)
        drain_inst.ins.sync_info = mybir.SyncInfo(
            on_wait=waits[:_MAX_WAITS], on_update=list(si.on_update)
        )
        for w in waits[_MAX_WAITS:]:
            n = self.nc.sync.nop(nofuse=True)
            n.ins.sync_info = mybir.SyncInfo(on_wait=[w], on_update=[])
    self.nc.all_engine_barrier()
    popped = self.nc._tile_sem_poison_stack.pop()
    assert popped is self._sem_poison
    self.nc.clear_and_free_semaphores(list(self.sems.allocated().values()))
    self.nc.all_engine_barrier()


tile_mod.TileContext._commit_instruction = _patched_commit
tile_mod.TileContext._drain_and_barrier = _patched_drain_and_barrier

# ---------------------------------------------------------------------------

B, S, D, H = 4, 2048, 1024, 16
DH = D // H + 1          # 65
P = H * DH               # 1040
HPC = H // 2             # heads per core
PC = HPC * DH            # 520, per-core P slice
N_CORES = 8

MT = S // 128            # 16 row blocks / k tiles
KT = 16                  # k tiles per attention
QB = 4                   # q blocks of 512
QW = 512
RKT = 2                  # k-tiles per score round (2 banks, double-buffered)
NR = KT // RKT           # 8 rounds

F32 = mybir.dt.float32
F32R = mybir.dt.float32r
BF16 = mybir.dt.bfloat16
BF = ml_dtypes.bfloat16

_BUILT = {}


def _build_nc():
    nc = bass.Bass("TRN2", target_bir_lowering=False, debug=False,
                   num_devices=N_CORES)

    xq_d = nc.dram_tensor("xq", [D, S], BF16, kind="ExternalInput").ap()
    xk_d = nc.dram_tensor("xk", [D, S], BF16, kind="ExternalInput").ap()
    xv_d = nc.dram_tensor("xv", [D, S], BF16, kind="ExternalInput").ap()
    # maskH[qb, p, j*QW+q] = maskT[j*128+p, qb*512+q] (multiplicative 0/1)
    mh = nc.dram_tensor("maskH", [QB, 128, KT * QW], BF16,
                        kind="ExternalInput").ap()
    wq_d = nc.dram_tensor("wqT", [D, PC], BF16, kind="ExternalInput").ap()
    wk_d = nc.dram_tensor("wkT", [D, PC], BF16, kind="ExternalInput").ap()
    wv_d = nc.dram_tensor("wvT", [D, PC], BF16, kind="ExternalInput").ap()
    bqT_d = nc.dram_tensor("bqT", [DH, HPC], F32, kind="ExternalInput").ap()
    bkT_d = nc.dram_tensor("bkT", [DH, HPC], F32, kind="ExternalInput").ap()
    bv_d = nc.dram_tensor("bv", [1, PC], BF16, kind="ExternalInput").ap()
    wo_d = nc.dram_tensor("woT", [PC, D], BF16, kind="ExternalInput").ap()
    sel8_d = nc.dram_tensor("sel8", [HPC, HPC * DH], F32R,
                            kind="ExternalInput").ap()
    out = nc.dram_tensor("out", [S, D], BF16, kind="ExternalOutput").ap()

    # packed 128-row k-tile ranges of the 520-row concatT / WoT
    PKT = [(0, 128), (128, 256), (256, 384), (384, 512), (512, 520)]

    with tile_mod.TileContext(nc) as tc:
        with tc.tile_pool(name="const", bufs=1) as pconst, \
             tc.tile_pool(name="qkT", bufs=1) as pqkT, \
             tc.tile_pool(name="vh", bufs=MT) as pvh, \
             tc.tile_pool(name="pwq", bufs=8) as pwq, \
             tc.tile_pool(name="pbq", bufs=1) as pbq, \
             tc.tile_pool(name="pxc", bufs=10) as pxc:

            ones_col = pconst.tile([1, 128], BF16, tag="ones")
            nc.gpsimd.memset(ones_col[:], 1.0)
            sel8 = pconst.tile([HPC, HPC * DH], F32R, tag="sel8")

            # [65, proj(q=0,k=1), head, S] -- dh on partitions
            qkT = pqkT.tile([DH, 2, HPC, S], BF16, tag="qkT")
            # v k-tiles with trailing ones column: [128, head, 65+1]
            vh = [pvh.tile([128, HPC, DH + 1], BF16, tag="vh", name=f"vh{j}")
                  for j in range(MT)]
            for j in range(MT):
                nc.gpsimd.memset(vh[j][:, :, DH:DH + 1], 1.0)

            def load_xq_chunk(qc):
                xts = []
                for d in range(8):
                    xt = pxc.tile([128, QW], BF16, tag="xqc",
                                  name=f"xqc{qc}_{d}")
                    nc.sync.dma_start(
                        xt[:], xq_d[d * 128:(d + 1) * 128,
                                    qc * QW:(qc + 1) * QW])
                    xts.append(xt)
                return xts

            def tproj_head(pj, pi, h, sblk, xts, wts, bT, xoff=0):
                """Transposed projection: qkT[0:65, pi, h, sblk] block."""
                ps = pj.tile([128, QW], F32, tag="psp",
                             name=f"tp{pi}_{h}_{sblk}")
                hs = slice(h * DH, (h + 1) * DH)
                xlo = sblk * QW - xoff
                for d in range(8):
                    nc.tensor.matmul(ps[0:DH, :], wts[d][:, hs],
                                     xts[d][:, xlo:xlo + QW],
                                     start=(d == 0), stop=(d == 7))
                nc.vector.tensor_scalar_add(
                    qkT[:, pi, h, sblk * QW:(sblk + 1) * QW], ps[0:DH, :],
                    bT[:, h:h + 1])

            # ---------------- phase A: projections --------------------------
            with tc.tile_pool(name="pxk", bufs=8) as pxk, \
                 tc.tile_pool(name="pxv", bufs=8) as pxv, \
                 tc.tile_pool(name="pwkv", bufs=9) as pwkv, \
                 tc.tile_pool(name="pbkv", bufs=2) as pbkv, \
                 tc.tile_pool(name="pj", bufs=4, space="PSUM") as pj, \
                 tc.tile_pool(name="psV", bufs=3, space="PSUM") as psV:

                xk_t, xv_t, wk_t, wv_t = [], [], [], []
                for d in range(8):
                    xt = pxk.tile([128, S], BF16, tag="xk", name=f"xk{d}")
                    eng = nc.sync if d % 2 == 0 else nc.scalar
                    eng.dma_start(xt[:], xk_d[d * 128:(d + 1) * 128, :])
                    xk_t.append(xt)
                for d in range(8):
                    wt = pwkv.tile([128, PC], BF16, tag="wk", name=f"wk{d}")
                    nc.gpsimd.dma_start(wt[:], wk_d[d * 128:(d + 1) * 128, :])
                    wk_t.append(wt)
                bkT = pbkv.tile([DH, HPC], F32, tag="bkT")
                nc.gpsimd.dma_start(bkT[:], bkT_d[:])
                for d in range(8):
                    xt = pxv.tile([128, S], BF16, tag="xv", name=f"xv{d}")
                    eng = nc.sync if d % 2 == 0 else nc.scalar
                    eng.dma_start(xt[:], xv_d[d * 128:(d + 1) * 128, :])
                    xv_t.append(xt)
                for d in range(8):
                    wt = pwkv.tile([128, HPC, DH], BF16, tag="wv",
                                   name=f"wv{d}")
                    nc.gpsimd.dma_start(
                        wt[:], wv_d[d * 128:(d + 1) * 128, :])
                    wv_t.append(wt)
                bv_t = pbkv.tile([1, HPC, DH], BF16, tag="bv")
                nc.gpsimd.dma_start(bv_t[:], bv_d[:])
                nc.gpsimd.dma_start(sel8[:], sel8_d[:])

                wq_t = []
                for d in range(8):
                    wt = pwq.tile([128, PC], BF16, tag="wq", name=f"wq{d}")
                    nc.scalar.dma_start(wt[:],
                                        wq_d[d * 128:(d + 1) * 128, :])
                    wq_t.append(wt)
                bqT = pbq.tile([DH, HPC], F32, tag="bqT")
                nc.gpsimd.dma_start(bqT[:], bqT_d[:])
                xq_c0 = load_xq_chunk(0)

                # K projection (transposed), then Q chunk 0
                for h in range(HPC):
                    for sblk in range(4):
                        tproj_head(pj, 1, h, sblk, xk_t, wk_t, bkT)
                for h in range(HPC):
                    tproj_head(pj, 0, h, 0, xq_c0, wq_t, bqT, xoff=0)

                # V projection (row-major into vh, bias via ones-row matmul)
                for m in range(MT):
                    for half in range(2):
                        hs = half * 4
                        ps = psV.tile([128, 4, DH], F32, tag="psV",
                                      name=f"pv{m}_{half}")
                        nc.tensor.matmul(ps[:], ones_col[0:1, :],
                                         bv_t[0:1, hs:hs + 4, :],
                                         start=True, stop=False)
                        for d in range(8):
                            nc.tensor.matmul(
                                ps[:], xv_t[d][:, m * 128:(m + 1) * 128],
                                wv_t[d][:, hs:hs + 4, :],
                                start=False, stop=(d == 7))
                        nc.vector.tensor_copy(vh[m][:, hs:hs + 4, 0:DH],
                                              ps[:])

            # ---------------- phase B: attention + out-projection -----------
            with tc.tile_pool(name="pm", bufs=4) as pm, \
                 tc.tile_pool(name="pp", bufs=6) as pp, \
                 tc.tile_pool(name="pc", bufs=1) as pc, \
                 tc.tile_pool(name="pwo", bufs=1) as pwo, \
                 tc.tile_pool(name="po", bufs=2) as po, \
                 tc.tile_pool(name="pt2", bufs=4) as pt2, \
                 tc.tile_pool(name="psS", bufs=2, space="PSUM") as psS, \
                 tc.tile_pool(name="psA", bufs=2, space="PSUM") as psA, \
                 tc.tile_pool(name="psp", bufs=2, space="PSUM") as psp:

                # packed concatT: 128-row tiles covering rows 0..520
                ccp = [pc.tile([b - a, S], BF16, tag=f"ccp{i}",
                               name=f"ccp{i}")
                       for i, (a, b) in enumerate(PKT)]
                wop = []
                for i, (a, b) in enumerate(PKT):
                    w = pwo.tile([b - a, D], BF16, tag=f"wop{i}",
                                 name=f"wop{i}")
                    nc.scalar.dma_start(w[:], wo_d[a:b, :])
                    wop.append(w)

                inv_sqrt = 1.0 / math.sqrt(float(DH))
                state = {}
                rnd = [0]  # global round counter for DVE/Pool mask split

                def load_masks(qb):
                    mts = []
                    for hf in range(2):
                        mt = pm.tile([128, KT // 2, QW], BF16, tag="mask",
                                     name=f"mask{qb}_{hf}")
                        nc.sync.dma_start(
                            mt[:], mh[qb, :, hf * (KT // 2) * QW:
                                      (hf + 1) * (KT // 2) * QW])
                        mts.append(mt)
                    return mts

                def attn_head(qb, h, mts, rsall, uovs, fillers=()):
                    """One head's rounds; fillers are emitted between rounds
                    so interleaved PE work lands in exp/mask wait gaps
                    instead of starving ACT at head boundaries."""
                    fillers = list(fillers)
                    ov = psA.tile([128, QW], F32, tag="psA",
                                  name=f"ov{qb}_{h}")
                    pts = []

                    def av_round(r):
                        pt = pts[r]
                        for jj in range(RKT):
                            j = r * RKT + jj
                            nc.tensor.matmul(
                                ov[0:DH + 1, :], vh[j][:, h, :],
                                pt[:, jj, :],
                                start=(j == 0), stop=(j == KT - 1))

                    for r in range(NR):
                        ss = psS.tile([128, RKT, QW], F32, tag="psS",
                                      name=f"ss{qb}_{h}_{r}")
                        for jj in range(RKT):
                            j = r * RKT + jj
                            nc.tensor.matmul(
                                ss[:, jj, :],
                                qkT[:, 1, h, j * 128:(j + 1) * 128],
                                qkT[:, 0, h, qb * QW:(qb + 1) * QW],
                                start=True, stop=True)
                        pt = pp.tile([128, RKT, QW], BF16, tag="pT",
                                     name=f"pt{qb}_{h}_{r}")
                        nc.scalar.activation(
                            pt[:], ss[:],
                            mybir.ActivationFunctionType.Exp,
                            scale=inv_sqrt)
                        mt = mts[r // (NR // 2)]
                        rr = r % (NR // 2)
                        nc.vector.tensor_mul(
                            pt[:], pt[:], mt[:, rr * RKT:(rr + 1) * RKT, :])
                        pts.append(pt)
                        if r >= 2:
                            av_round(r - 2)
                        if r >= 2 and fillers:
                            fillers.pop(0)()
                    while fillers:
                        fillers.pop(0)()
                    av_round(NR - 2)
                    av_round(NR - 1)
                    uov = pt2.tile([DH + 1, QW], BF16, tag="uov",
                                   name=f"uov{qb}_{h}", bufs=12)
                    nc.vector.tensor_copy(uov[:], ov[0:DH + 1, :])
                    nc.sync.dma_start(rsall[h:h + 1, :],
                                      uov[DH:DH + 1, :])
                    uovs.append(uov)

                def tproj_pieces(pi, hh, sblk, xts, bT, xoff):
                    box = {}

                    def p1():
                        ps = psp.tile([128, QW], F32, tag="psp",
                                      name=f"tpp{pi}_{hh}_{sblk}")
                        box["ps"] = ps
                        hsl = slice(hh * DH, (hh + 1) * DH)
                        xlo = sblk * QW - xoff
                        for d in range(4):
                            nc.tensor.matmul(ps[0:DH, :], wq_t[d][:, hsl],
                                             xts[d][:, xlo:xlo + QW],
                                             start=(d == 0), stop=False)

                    def p2():
                        ps = box["ps"]
                        hsl = slice(hh * DH, (hh + 1) * DH)
                        xlo = sblk * QW - xoff
                        for d in range(4, 8):
                            nc.tensor.matmul(ps[0:DH, :], wq_t[d][:, hsl],
                                             xts[d][:, xlo:xlo + QW],
                                             start=False, stop=(d == 7))
                        nc.vector.tensor_scalar_add(
                            qkT[:, pi, hh, sblk * QW:(sblk + 1) * QW],
                            ps[0:DH, :], bT[:, hh:hh + 1])

                    return [p1, p2]

                def outproj_pieces(m):
                    box = {}

                    def pn(n):
                        def run():
                            if n == 0:
                                box["osb"] = po.tile([128, D], BF16,
                                                     tag="osb",
                                                     name=f"osb{m}")
                            osb = box["osb"]
                            ps = psp.tile([128, QW], F32, tag="psp",
                                          name=f"psop{m}_{n}")
                            for i, (a, b) in enumerate(PKT):
                                nc.tensor.matmul(
                                    ps[:], ccp[i][:, m * 128:(m + 1) * 128],
                                    wop[i][:, n * QW:(n + 1) * QW],
                                    start=(i == 0),
                                    stop=(i == len(PKT) - 1))
                            nc.vector.tensor_copy(
                                osb[:, n * QW:(n + 1) * QW], ps[:])
                            if n == 1:
                                nc.sync.dma_start(
                                    out[m * 128:(m + 1) * 128, :], osb[:])
                        return run

                    return [pn(0), pn(1)]

                def compute_recip(qb, lo, hi, c0=0, c1=QW):
                    st = state[qb]
                    if "rcall" not in st:
                        st["rcall"] = pt2.tile([HPC, QW], F32R, tag="rcall",
                                               name=f"rcall{qb}", bufs=2)
                    with nc.allow_low_precision(
                            reason="f32r reciprocal feeds 1-pass f32r "
                                   "broadcast matmul; 2^-11 rel is ample"):
                        nc.vector.reciprocal(st["rcall"][lo:hi, c0:c1],
                                             st["rsall"][lo:hi, c0:c1])

                def norm_heads(qb, hs):
                    st = state[qb]
                    if "rcall" not in st:
                        compute_recip(qb, 0, HPC)
                    rcall = st["rcall"]
                    for h in hs:
                        rbp = psp.tile([128, QW], F32, tag="psp",
                                       name=f"rbp{qb}_{h}")
                        nc.tensor.matmul(rbp[0:DH, :],
                                         sel8[:, h * DH:(h + 1) * DH],
                                         rcall[:],
                                         start=True, stop=True)
                        cch = pt2.tile([DH, QW], BF16, tag="cch",
                                       name=f"cch{qb}_{h}", bufs=2)
                        nc.vector.tensor_mul(cch[:], rbp[0:DH, :],
                                             st["uovs"][h][0:DH, :])
                        # pack into 128-row concatT tiles (DMA shifts rows)
                        r0 = h * DH
                        for i, (a, b) in enumerate(PKT):
                            lo, hi = max(r0, a), min(r0 + DH, b)
                            if lo < hi:
                                nc.gpsimd.dma_start(
                                    ccp[i][lo - a:hi - a,
                                           qb * QW:(qb + 1) * QW],
                                    cch[lo - r0:hi - r0, :])

                def outproj_m(m):
                    osb = po.tile([128, D], BF16, tag="osb")
                    for n in range(2):
                        ps = psp.tile([128, QW], F32, tag="psp",
                                      name=f"psop{m}_{n}")
                        for i, (a, b) in enumerate(PKT):
                            nc.tensor.matmul(
                                ps[:], ccp[i][:, m * 128:(m + 1) * 128],
                                wop[i][:, n * QW:(n + 1) * QW],
                                start=(i == 0), stop=(i == len(PKT) - 1))
                        nc.vector.tensor_copy(osb[:, n * QW:(n + 1) * QW],
                                              ps[:])
                    nc.sync.dma_start(out[m * 128:(m + 1) * 128, :],
                                      osb[:])

                mts = load_masks(0)
                for qb in range(QB):
                    nmts = load_masks(qb + 1) if qb < QB - 1 else None
                    rsall = pt2.tile([HPC, QW], BF16, tag="rsall",
                                     name=f"rsall{qb}", bufs=2)
                    uovs = []
                    state[qb] = {"rsall": rsall, "uovs": uovs}
                    last = qb == QB - 1
                    for h in range(HPC):
                        if last and h == 5:
                            compute_recip(qb, 0, 5)
                        fill = []
                        if qb > 0 and h == 0:
                            for ci in range(1, 4):
                                fill.append(
                                    lambda q_=qb - 1, c_=ci: compute_recip(
                                        q_, 0, HPC, c_ * QW // 4,
                                        (c_ + 1) * QW // 4))
                            if qb >= 2:
                                fill += outproj_pieces((qb - 2) * 4 + 3)
                        if qb < QB - 1:
                            if h == 0:
                                state["xq"] = load_xq_chunk(qb + 1)
                            fill += tproj_pieces(0, h, qb + 1, state["xq"],
                                                 bqT, (qb + 1) * QW)
                        if qb > 0:
                            if 1 <= h <= 4:
                                fill.append(
                                    lambda q_=qb - 1, a_=2 * (h - 1):
                                    norm_heads(q_, [a_]))
                                fill.append(
                                    lambda q_=qb - 1, a_=2 * (h - 1) + 1:
                                    norm_heads(q_, [a_]))
                            elif h >= 5:
                                fill += outproj_pieces(
                                    (qb - 1) * 4 + (h - 5))
                                if last and h == 5:
                                    fill += outproj_pieces((QB - 2) * 4 + 3)
                        if last and h == 6:
                            for hh in range(0, 3):
                                fill.append(
                                    lambda a_=hh: norm_heads(qb, [a_]))
                        elif last and h == 7:
                            fill.insert(0, lambda: compute_recip(
                                qb, 0, 7, QW // 2, QW))
                            fill.insert(0, lambda: compute_recip(
                                qb, 0, 7, 0, QW // 2))
                            for hh in range(3, 7):
                                fill.append(
                                    lambda a_=hh: norm_heads(qb, [a_]))
                        attn_head(qb, h, mts, rsall, uovs, fill)
                    if not last:
                        # first reciprocal piece at the boundary; the rest
                        # interleave as fillers so neither PE nor DVE stalls
                        compute_recip(qb, 0, HPC, 0, QW // 4)
                    mts = nmts
                compute_recip(QB - 1, 0, HPC)
                norm_heads(QB - 1, range(7, HPC))
                for m in range((QB - 1) * 4, QB * 4):
                    for p in outproj_pieces(m):
                        p()

    return nc


F8NP = ml_dtypes.float8_e4m3


def pack8(xT, scale):
    """[D, N] -> [4, 128, 2, N] fp8 with d-chunk pairs interleaved."""
    D_, N_ = xT.shape
    y = (xT * scale).reshape(4, 2, 128, N_).transpose(0, 2, 1, 3)
    assert np.abs(y).max() < 240.0
    return np.ascontiguousarray(y).astype(F8NP)


def pack8w(wT, scale):
    """[D, PC] -> [4, 128, 2, HPC, 80] fp8, heads padded 65 -> 80."""
    D_, _ = wT.shape
    y = np.zeros((4, 2, 128, HPC, 80), np.float32)
    w = (wT * scale).reshape(4, 2, 128, HPC, DH)
    assert np.abs(w).max() < 240.0
    y[..., 0:DH] = w
    return np.ascontiguousarray(y.transpose(0, 2, 1, 3, 4)).astype(F8NP)


def _prep_inputs(q, k, v, mask, Wq, bqv, Wk, bkv, Wv, bvv, Wo):
    """Per-core input maps (numpy, host-side shard + cast)."""
    in_maps = []
    sel8 = np.zeros((HPC, HPC * DH), np.float32)
    for h in range(HPC):
        sel8[h, h * DH:(h + 1) * DH] = 1.0
    mask_h = {}
    for b in range(B):
        mt = (mask[b, 0] != 0).astype(np.float32).T  # [k, q]
        m4 = mt.reshape(KT, 128, QB, QW).transpose(2, 1, 0, 3)
        mask_h[b] = np.ascontiguousarray(m4.reshape(QB, 128, KT * QW)).astype(BF)
    for c in range(N_CORES):
        b, hh = c // 2, c % 2
        sl = slice(hh * PC, (hh + 1) * PC)
        in_maps.append({
            "xq": np.ascontiguousarray(q[b].T).astype(BF),
            "xk": np.ascontiguousarray(k[b].T).astype(BF),
            "xv": np.ascontiguousarray(v[b].T).astype(BF),
            "maskH": mask_h[b],
            "wqT": np.ascontiguousarray(Wq[sl, :].T).astype(BF),
            "wkT": np.ascontiguousarray(Wk[sl, :].T).astype(BF),
            "wvT": np.ascontiguousarray(Wv[sl, :].T).astype(BF),
            "bqT": np.ascontiguousarray(
                bqv[sl].reshape(HPC, DH).T).astype(np.float32),
            "bkT": np.ascontiguousarray(
                bkv[sl].reshape(HPC, DH).T).astype(np.float32),
            "bv": bvv[sl].reshape(1, PC).astype(BF),
            "woT": np.ascontiguousarray(Wo[:, sl].T).astype(BF),
            "sel8": sel8,
        })
    return in_maps


def run_sharded(in_maps, **kwargs):
    if "nc" not in _BUILT:
        _BUILT["nc"] = _build_nc()
    return run_bass_kernel_spmd(_BUILT["nc"], in_maps,
                                core_ids=list(range(N_CORES)), **kwargs)


def kernel(q, k, v, mask, Wq, bq, Wk, bk, Wv, bv, Wo, bo):
    q = np.asarray(q, np.float32)
    k = np.asarray(k, np.float32)
    v = np.asarray(v, np.float32)
    mask = np.asarray(mask)
    in_maps = _prep_inputs(q, k, v, mask,
                           np.asarray(Wq, np.float32), np.asarray(bq, np.float32),
                           np.asarray(Wk, np.float32), np.asarray(bk, np.float32),
                           np.asarray(Wv, np.float32), np.asarray(bv, np.float32),
                           np.asarray(Wo, np.float32))
    res = run_sharded(in_maps)
    bo32 = np.asarray(bo, np.float32)
    out = np.empty((B, S, D), np.float32)
    for b in range(B):
        out[b] = (res.results[2 * b]["out"].astype(np.float32)
                  + res.results[2 * b + 1]["out"].astype(np.float32) + bo32)
    return out
